# revision 16
# baseline (speedup 1.0000x reference)
"""Additive (Bahdanau) attention scoring kernel for Trainium2, 8-core SPMD.

Reference computation (B=16, S=4096, D=1024, all fp32):
    q      = target @ Wq.T                    # [B, D]
    k      = memory @ Wk.T                    # [B, S, D]
    scores = tanh(q[:, None, :] + k) @ v      # [B, S]
    out    = softmax(scores - 1e9 * mask, axis=-1)

Sharding: batch across the 8 cores (2 batches per core), weights replicated.

Per-core device pipeline (all python-unrolled, Tile-scheduled):
  - k^T computed tile-by-tile as [e=128, s=512] PSUM tiles:
      matmul(lhsT=WkT[d,e] chunk, rhs=memT[d,s] chunk) accumulated over d.
    memory is fed pre-transposed from the host ([D, S] per batch) so the
    contraction dim d lands on SBUF partitions with clean DMA descriptors.
    Matmuls run as float32r (1 cycle/row at N=512, ~4x faster than plain
    fp32 at near-fp32 input precision).
  - One ACT pass fuses the q-add and tanh: tanh(k + q) with q as the
    per-partition bias vector.
  - v-dot on the PE: psum[1, 512] += v_chunk.T @ tanh_tile, accumulated
    over the 8 e-chunks; exp() applied in the ACT copy out of PSUM.
  - Softmax (no max-shift needed: |scores| <= sum|v| ~ 8, exp cannot
    overflow; masking is exp(s) * keep which matches the reference's
    exp(-1e9) == 0 exactly): exp strips bounce through DRAM into a
    [128, 32] layout, one fused DVE mul+reduce applies the mask and row
    partial sums, a ones-matmul reduces across partitions, reciprocal +
    per-partition scale finish the normalization.
"""

from contextlib import ExitStack

import numpy as np

import concourse.tile as tile
from concourse import bacc, mybir

B, S, D = 16, 4096, 1024
N_CORES = 8
NB = B // N_CORES  # batches per core
P = 128
DC = D // P        # contraction chunks
ET = D // P        # e tiles
SW = 512           # strip width along s
NSTRIP = S // SW
SQ = S // P        # 32: free dim of the [128, 32] softmax layout

F32 = mybir.dt.float32
F32R = mybir.dt.float32r
AF = mybir.ActivationFunctionType

_CACHE = {}


def _build_program(stage=3):
    """stage: 1 = dma+matmul+tanh only, 2 = +vdot/exp/scratch, 3 = full."""
    nc = bacc.Bacc("TRN2", target_bir_lowering=False, debug=False)

    memT = nc.dram_tensor("memT", [NB, D, S], F32, kind="ExternalInput").ap()
    wkT = nc.dram_tensor("wkT", [D, D], F32, kind="ExternalInput").ap()
    wqT = nc.dram_tensor("wqT", [D, D], F32, kind="ExternalInput").ap()
    tgtT = nc.dram_tensor("tgtT", [D, NB], F32, kind="ExternalInput").ap()
    vT = nc.dram_tensor("vT", [P, ET], F32, kind="ExternalInput").ap()
    keep = nc.dram_tensor("keep", [NB, P, SQ], F32, kind="ExternalInput").ap()
    out = nc.dram_tensor("out", [NB, P, SQ], F32, kind="ExternalOutput").ap()

    with tile.TileContext(nc) as tc, ExitStack() as ctx:
        consts = ctx.enter_context(tc.tile_pool(name="consts", bufs=1))
        mem_pool = ctx.enter_context(tc.tile_pool(name="mem", bufs=2))
        tt_pool = ctx.enter_context(tc.tile_pool(name="tt", bufs=4))
        strip_pool = ctx.enter_context(tc.tile_pool(name="strip", bufs=2))
        fin_pool = ctx.enter_context(tc.tile_pool(name="fin", bufs=2))
        kps_pool = ctx.enter_context(tc.tile_pool(name="kps", bufs=4, space="PSUM"))
        vd_pool = ctx.enter_context(tc.tile_pool(name="vd", bufs=2, space="PSUM"))
        sm_pool = ctx.enter_context(tc.tile_pool(name="smps", bufs=2, space="PSUM"))
        dram_pool = ctx.enter_context(tc.tile_pool(name="scratch", bufs=2, space="DRAM"))

        # --- small constants (cheap DMAs first) ---
        tgt_sb = consts.tile([P, DC * NB], F32)
        for dc in range(DC):
            nc.sync.dma_start(tgt_sb[:, dc * NB:(dc + 1) * NB], tgtT[dc * P:(dc + 1) * P, :])
        v_sb = consts.tile([P, ET], F32)
        nc.sync.dma_start(v_sb[:], vT[:, :])
        # fp32r matmul operands must be produced by a rounding compute op,
        # not a DMA (BIR verifier rule) — hence the _r copies below.
        v_r = consts.tile([P, ET], F32R)
        nc.vector.tensor_copy(v_r[:], v_sb[:])
        keep_sb = consts.tile([P, NB * SQ], F32)
        for b in range(NB):
            nc.sync.dma_start(keep_sb[:, b * SQ:(b + 1) * SQ], keep[b])
        ones_sb = consts.tile([P, P], F32)
        nc.vector.memset(ones_sb[:], 1.0)

        # --- weights (big DMAs; Wk first so the main matmul can start) ---
        wk_sb = consts.tile([P, DC * D], F32)
        wk_r = consts.tile([P, DC * D], F32R)
        for dc in range(DC):
            nc.sync.dma_start(wk_sb[:, dc * D:(dc + 1) * D], wkT[dc * P:(dc + 1) * P, :])
            nc.vector.tensor_copy(wk_r[:, dc * D:(dc + 1) * D], wk_sb[:, dc * D:(dc + 1) * D])
        wq_sb = consts.tile([P, DC * D], F32)
        for dc in range(DC):
            nc.sync.dma_start(wq_sb[:, dc * D:(dc + 1) * D], wqT[dc * P:(dc + 1) * P, :])

        q_sb = consts.tile([P, ET * NB], F32)

        # q^T[e, b] = sum_d Wq[e, d] * target[b, d], via 8 e-tiles.
        # Emitted first: under Tile, trace order is semantic order, and every
        # tanh below reads q_sb. The WqT DMA-wait hides under the WkT and
        # first-strip loads.
        for et in range(ET):
            q_ps = sm_pool.tile([P, NB], F32, tag="small", name="q_ps")
            for dc in range(DC):
                nc.tensor.matmul(
                    q_ps[:],
                    wq_sb[:, dc * D + et * P: dc * D + (et + 1) * P],
                    tgt_sb[:, dc * NB:(dc + 1) * NB],
                    start=(dc == 0),
                    stop=(dc == DC - 1),
                )
            nc.vector.tensor_copy(q_sb[:, et * NB:(et + 1) * NB], q_ps[:])

        def emit_vd(vd_ps, tts, c):
            nc.tensor.matmul(
                vd_ps[:],
                v_r[:, c:c + 1],
                tts[c][:],
                start=(c == 0),
                stop=(c == ET - 1),
            )

        for b in range(NB):
            scratch = dram_pool.tile([1, S], F32, tag="scr", name="scr")
            for sp in range(NSTRIP):
                mem_sb = mem_pool.tile([P, DC * SW], F32)
                mem_r = mem_pool.tile([P, DC * SW], F32R, tag="mem_r", name="mem_r")
                for dc in range(DC):
                    nc.sync.dma_start(
                        mem_sb[:, dc * SW:(dc + 1) * SW],
                        memT[b, dc * P:(dc + 1) * P, sp * SW:(sp + 1) * SW],
                    )
                    nc.vector.tensor_copy(
                        mem_r[:, dc * SW:(dc + 1) * SW], mem_sb[:, dc * SW:(dc + 1) * SW]
                    )
                vd_ps = vd_pool.tile([1, SW], F32, tag="vd", name="vd_ps")
                tts = []
                for et in range(ET):
                    k_ps = kps_pool.tile([P, SW], F32, tag="k", name="k_ps")
                    for dc in range(DC):
                        nc.tensor.matmul(
                            k_ps[:],
                            wk_r[:, dc * D + et * P: dc * D + (et + 1) * P],
                            mem_r[:, dc * SW:(dc + 1) * SW],
                            start=(dc == 0),
                            stop=(dc == DC - 1),
                        )
                    tt = tt_pool.tile([P, SW], F32R, tag="tt", name="tt")
                    nc.scalar.activation(
                        tt[:], k_ps[:], AF.Tanh,
                        bias=q_sb[:, et * NB + b: et * NB + b + 1],
                    )
                    tts.append(tt)
                    # keep the PE stream 2 e-tiles ahead of the v-dot so it
                    # never stalls waiting on the ACT tanh
                    if stage >= 2 and et >= 2:
                        emit_vd(vd_ps, tts, et - 2)
                if stage < 2:
                    if sp == NSTRIP - 1:
                        dbg = fin_pool.tile([P, SQ], F32, tag="outt", name="dbg")
                        nc.vector.tensor_copy(dbg[:], tts[7][:, :SQ])
                        nc.sync.dma_start(out[b], dbg[:])
                    continue
                emit_vd(vd_ps, tts, ET - 2)
                emit_vd(vd_ps, tts, ET - 1)

                strip_sb = strip_pool.tile([1, SW], F32, tag="strip", name="strip_sb")
                nc.scalar.activation(strip_sb[:], vd_ps[:], AF.Exp)
                nc.sync.dma_start(scratch[:, sp * SW:(sp + 1) * SW], strip_sb[:])

            if stage < 2:
                continue
            # --- masked softmax finale for batch b ---
            esq = fin_pool.tile([P, SQ], F32, tag="esq", name="esq")
            nc.sync.dma_start(esq[:], scratch.rearrange("o (p f) -> (o p) f", p=P))
            if stage < 3:
                outt = fin_pool.tile([P, SQ], F32, tag="outt", name="outt")
                nc.vector.tensor_copy(outt[:], esq[:])
                nc.sync.dma_start(out[b], outt[:])
                continue
            em = fin_pool.tile([P, SQ], F32, tag="em", name="em")
            part = fin_pool.tile([P, 1], F32, tag="part", name="part")
            nc.vector.tensor_mul(em[:], esq[:], keep_sb[:, b * SQ:(b + 1) * SQ])
            nc.vector.reduce_sum(part[:], em[:], axis=mybir.AxisListType.X)
            if stage < 26:  # stage 25: stop after the fused mul+reduce
                outt = fin_pool.tile([P, SQ], F32, tag="outt", name="outt")
                nc.vector.tensor_copy(outt[:], em[:])
                nc.sync.dma_start(out[b], outt[:])
                continue
            tot_ps = sm_pool.tile([P, 1], F32, tag="small", name="tot_ps")
            nc.tensor.matmul(tot_ps[:], ones_sb[:], part[:], start=True, stop=True)
            recip = fin_pool.tile([P, 1], F32, tag="recip", name="recip")
            nc.vector.reciprocal(recip[:], tot_ps[:])
            if stage < 27:  # stage 26: stop after partition-sum + reciprocal
                outt = fin_pool.tile([P, SQ], F32, tag="outt", name="outt")
                nc.vector.tensor_copy(outt[:], em[:])
                nc.vector.tensor_copy(outt[:, 0:1], recip[:])
                nc.sync.dma_start(out[b], outt[:])
                continue
            outt = fin_pool.tile([P, SQ], F32, tag="outt", name="outt")
            nc.vector.tensor_scalar_mul(outt[:], em[:], recip[:, 0:1])
            nc.sync.dma_start(out[b], outt[:])

    nc.compile()
    return nc


def get_program(stage=None):
    import os

    if stage is None:
        stage = int(os.environ.get("KERNEL_STAGE", "27"))
    if stage not in _CACHE:
        _CACHE[stage] = _build_program(stage)
    return _CACHE[stage]


def prepare_in_maps(memory, target, memory_mask, Wq, Wk, v):
    memory = np.asarray(memory, dtype=np.float32)
    target = np.asarray(target, dtype=np.float32)
    Wq = np.asarray(Wq, dtype=np.float32)
    Wk = np.asarray(Wk, dtype=np.float32)
    v = np.asarray(v, dtype=np.float32)
    mask = np.asarray(memory_mask)

    # host-side sharding / layout prep
    memT = np.ascontiguousarray(memory.transpose(0, 2, 1))          # [B, D, S]
    wkT = np.ascontiguousarray(Wk.T)                                # [D, D]
    wqT = np.ascontiguousarray(Wq.T)                                # [D, D]
    tgtT = np.ascontiguousarray(target.T)                           # [D, B]
    vT = np.ascontiguousarray(v.reshape(ET, P).T)                   # [P, ET]
    keep = np.ascontiguousarray(
        (~mask).astype(np.float32).reshape(B, P, SQ))               # [B, P, SQ]

    return [
        {
            "memT": np.ascontiguousarray(memT[c * NB:(c + 1) * NB]),
            "wkT": wkT,
            "wqT": wqT,
            "tgtT": np.ascontiguousarray(tgtT[:, c * NB:(c + 1) * NB]),
            "vT": vT,
            "keep": np.ascontiguousarray(keep[c * NB:(c + 1) * NB]),
        }
        for c in range(N_CORES)
    ]


def gather_output(results):
    out = np.empty((B, S), dtype=np.float32)
    for c in range(N_CORES):
        out[c * NB:(c + 1) * NB] = results[c]["out"].reshape(NB, S)
    return out


def kernel(memory, target, memory_mask, Wq, Wk, v):
    from concourse.bass_utils import run_bass_kernel_spmd

    in_maps = prepare_in_maps(memory, target, memory_mask, Wq, Wk, v)
    nc = get_program()
    res = run_bass_kernel_spmd(nc, in_maps, list(range(N_CORES)))
    return gather_output(res.results)


# revision 31
# speedup vs baseline: 1.0622x; 1.0622x over previous
"""Additive (Bahdanau) attention scoring kernel for Trainium2, 8-core SPMD.

Reference computation (B=16, S=4096, D=1024, all fp32):
    q      = target @ Wq.T                    # [B, D]
    k      = memory @ Wk.T                    # [B, S, D]
    scores = tanh(q[:, None, :] + k) @ v      # [B, S]
    out    = softmax(scores - 1e9 * mask, axis=-1)

Sharding: batch across the 8 cores (2 batches per core), weights replicated.

Per-core device pipeline (all python-unrolled, Tile-scheduled):
  - k^T computed tile-by-tile as [e=128, s=512] PSUM tiles:
      matmul(lhsT=WkT[d,e] chunk, rhs=memT[d,s] chunk) accumulated over d.
    memory is fed pre-transposed from the host ([D, S] per batch) so the
    contraction dim d lands on SBUF partitions with clean DMA descriptors.
    Matmuls run as float32r (1 cycle/row at N=512, ~4x faster than plain
    fp32 at near-fp32 input precision).
  - One ACT pass fuses the q-add and tanh: tanh(k + q) with q as the
    per-partition bias vector.
  - v-dot on the PE: psum[1, 512] += v_chunk.T @ tanh_tile, accumulated
    over the 8 e-chunks; exp() applied in the ACT copy out of PSUM.
  - Softmax (no max-shift needed: |scores| <= sum|v| ~ 8, exp cannot
    overflow; masking is exp(s) * keep which matches the reference's
    exp(-1e9) == 0 exactly): exp strips bounce through DRAM into a
    [128, 32] layout, one fused DVE mul+reduce applies the mask and row
    partial sums, a ones-matmul reduces across partitions, reciprocal +
    per-partition scale finish the normalization.
"""

from contextlib import ExitStack

import numpy as np

import concourse.tile as tile
from concourse import bacc, mybir

B, S, D = 16, 4096, 1024
N_CORES = 8
NB = B // N_CORES  # batches per core
P = 128
DC = D // P        # contraction chunks
ET = D // P        # e tiles
SW = 512           # strip width along s
NSTRIP = S // SW
SQ = S // P        # 32: free dim of the [128, 32] softmax layout

F32 = mybir.dt.float32
F32R = mybir.dt.float32r
AF = mybir.ActivationFunctionType

_CACHE = {}


def _build_program(stage=3):
    """stage: 1 = dma+matmul+tanh only, 2 = +vdot/exp/scratch, 3 = full."""
    nc = bacc.Bacc("TRN2", target_bir_lowering=False, debug=False)

    memT = nc.dram_tensor("memT", [NB, D, S], F32, kind="ExternalInput").ap()
    wkT = nc.dram_tensor("wkT", [D, D], F32, kind="ExternalInput").ap()
    wqT = nc.dram_tensor("wqT", [D, D], F32, kind="ExternalInput").ap()
    tgtT = nc.dram_tensor("tgtT", [D, NB], F32, kind="ExternalInput").ap()
    vT = nc.dram_tensor("vT", [P, ET], F32, kind="ExternalInput").ap()
    keep = nc.dram_tensor("keep", [NB, P, SQ], F32, kind="ExternalInput").ap()
    out = nc.dram_tensor("out", [NB, P, SQ], F32, kind="ExternalOutput").ap()

    with tile.TileContext(nc) as tc, ExitStack() as ctx:
        consts = ctx.enter_context(tc.tile_pool(name="consts", bufs=1))
        mem_pool = ctx.enter_context(tc.tile_pool(name="mem", bufs=2))
        tt_pool = ctx.enter_context(tc.tile_pool(name="tt", bufs=4))
        strip_pool = ctx.enter_context(tc.tile_pool(name="strip", bufs=2))
        fin_pool = ctx.enter_context(tc.tile_pool(name="fin", bufs=2))
        kps_pool = ctx.enter_context(tc.tile_pool(name="kps", bufs=4, space="PSUM"))
        vd_pool = ctx.enter_context(tc.tile_pool(name="vd", bufs=2, space="PSUM"))
        sm_pool = ctx.enter_context(tc.tile_pool(name="smps", bufs=2, space="PSUM"))
        dram_pool = ctx.enter_context(tc.tile_pool(name="scratch", bufs=2, space="DRAM"))

        # fp32r matmul operands must be produced by a rounding compute op
        # (every writer of the location — so no in-place tricks, the DMA
        # landing buffer and the rounded tile must be separate memory).

        # --- small constants (cheap DMAs first) ---
        tgt_sb = consts.tile([P, DC * NB], F32)
        for dc in range(DC):
            nc.sync.dma_start(tgt_sb[:, dc * NB:(dc + 1) * NB], tgtT[dc * P:(dc + 1) * P, :])
        tgt_r = consts.tile([P, DC * NB], F32R)
        nc.vector.tensor_copy(tgt_r[:], tgt_sb[:])
        v_sb = consts.tile([P, ET], F32)
        nc.sync.dma_start(v_sb[:], vT[:, :])
        v_r = consts.tile([P, ET], F32R)
        nc.vector.tensor_copy(v_r[:], v_sb[:])
        keep_sb = consts.tile([P, NB * SQ], F32)
        for b in range(NB):
            nc.sync.dma_start(keep_sb[:, b * SQ:(b + 1) * SQ], keep[b])
        ones_sb = consts.tile([P, P], F32)
        nc.vector.memset(ones_sb[:], 1.0)

        # --- weights: Wq first (the q matmuls below are first in PE order),
        # then Wk. The two f32 landing buffers share one pool slot (their
        # lifetimes are sequential) to stay inside SBUF.
        QR = int(__import__("os").environ.get("KERNEL_QF32R", "1"))
        wq_sb = consts.tile([P, DC * D], F32, tag="wstage", name="wq_sb")
        if QR:
            wq_r = consts.tile([P, DC * D], F32R)
        for dc in range(DC):
            nc.sync.dma_start(wq_sb[:, dc * D:(dc + 1) * D], wqT[dc * P:(dc + 1) * P, :])
            if QR:
                nc.vector.tensor_copy(wq_r[:, dc * D:(dc + 1) * D], wq_sb[:, dc * D:(dc + 1) * D])
        wk_r = consts.tile([P, DC * D], F32R)
        wk_sb = consts.tile([P, DC * D], F32, tag="wstage", name="wk_sb")
        for dc in range(DC):
            nc.sync.dma_start(wk_sb[:, dc * D:(dc + 1) * D], wkT[dc * P:(dc + 1) * P, :])
            nc.vector.tensor_copy(wk_r[:, dc * D:(dc + 1) * D], wk_sb[:, dc * D:(dc + 1) * D])

        q_sb = consts.tile([P, ET * NB], F32)

        # q[b, e] = sum_d target[b, d] * Wq[e, d]. Emitted first: under Tile,
        # trace order is semantic order, and every tanh below reads q_sb.
        if QR:
            # fp32r with target as the M=2 stationary and WqT as the N=512
            # moving operand (fp32r needs N>=256: small-N fp32r matmuls hard-
            # fault the device). Result lands as [2, 1024]; bounce it through
            # DRAM to transpose into the per-partition bias layout [128, 16].
            q_row = consts.tile([NB, D], F32)
            for j in range(D // SW):
                q_ps2 = sm_pool.tile([NB, SW], F32, tag="small", name="q_ps2")
                for dc in range(DC):
                    nc.tensor.matmul(
                        q_ps2[:],
                        tgt_r[:, dc * NB:(dc + 1) * NB],
                        wq_r[:, dc * D + j * SW: dc * D + (j + 1) * SW],
                        start=(dc == 0),
                        stop=(dc == DC - 1),
                    )
                nc.vector.tensor_copy(q_row[:, j * SW:(j + 1) * SW], q_ps2[:])
            qscr = dram_pool.tile([NB, D], F32, tag="qscr", name="qscr")
            nc.sync.dma_start(qscr[:], q_row[:])
            for b in range(NB):
                nc.sync.dma_start(
                    q_sb[:, b * ET:(b + 1) * ET],
                    qscr[b].rearrange("(et p) -> p et", p=P),
                )
        else:
            # plain-fp32 fallback: one [e_tile, b] matmul group per e-tile
            for et in range(ET):
                q_ps = sm_pool.tile([P, NB], F32, tag="small", name="q_ps")
                for dc in range(DC):
                    nc.tensor.matmul(
                        q_ps[:],
                        wq_sb[:, dc * D + et * P: dc * D + (et + 1) * P],
                        tgt_sb[:, dc * NB:(dc + 1) * NB],
                        start=(dc == 0),
                        stop=(dc == DC - 1),
                    )
                for b in range(NB):
                    nc.vector.tensor_copy(q_sb[:, b * ET + et: b * ET + et + 1], q_ps[:, b:b + 1])

        def emit_vd(vd_ps, tts, c):
            nc.tensor.matmul(
                vd_ps[:],
                v_r[:, c:c + 1],
                tts[c][:],
                start=(c == 0),
                stop=(c == ET - 1),
            )

        for b in range(NB):
            scratch = dram_pool.tile([1, S], F32, tag="scr", name="scr")
            for sp in range(NSTRIP):
                mem_sb = mem_pool.tile([P, DC * SW], F32)
                mem_r = mem_pool.tile([P, DC * SW], F32R, tag="mem_r", name="mem_r")
                for dc in range(DC):
                    nc.sync.dma_start(
                        mem_sb[:, dc * SW:(dc + 1) * SW],
                        memT[b, dc * P:(dc + 1) * P, sp * SW:(sp + 1) * SW],
                    )
                    nc.vector.tensor_copy(
                        mem_r[:, dc * SW:(dc + 1) * SW], mem_sb[:, dc * SW:(dc + 1) * SW]
                    )
                vd_ps = vd_pool.tile([1, SW], F32, tag="vd", name="vd_ps")
                tts = []
                for et in range(ET):
                    k_ps = kps_pool.tile([P, SW], F32, tag="k", name="k_ps")
                    for dc in range(DC):
                        nc.tensor.matmul(
                            k_ps[:],
                            wk_r[:, dc * D + et * P: dc * D + (et + 1) * P],
                            mem_r[:, dc * SW:(dc + 1) * SW],
                            start=(dc == 0),
                            stop=(dc == DC - 1),
                        )
                    tt = tt_pool.tile([P, SW], F32R, tag="tt", name="tt")
                    nc.scalar.activation(
                        tt[:], k_ps[:], AF.Tanh,
                        bias=q_sb[:, b * ET + et: b * ET + et + 1],
                    )
                    tts.append(tt)
                    # keep the PE stream 2 e-tiles ahead of the v-dot so it
                    # never stalls waiting on the ACT tanh
                    if stage >= 2 and et >= 2:
                        emit_vd(vd_ps, tts, et - 2)
                if stage < 2:
                    if sp == NSTRIP - 1:
                        dbg = fin_pool.tile([P, SQ], F32, tag="outt", name="dbg")
                        nc.vector.tensor_copy(dbg[:], tts[7][:, :SQ])
                        nc.sync.dma_start(out[b], dbg[:])
                    continue
                emit_vd(vd_ps, tts, ET - 2)
                emit_vd(vd_ps, tts, ET - 1)

                strip_sb = strip_pool.tile([1, SW], F32, tag="strip", name="strip_sb")
                nc.scalar.activation(strip_sb[:], vd_ps[:], AF.Exp)
                nc.sync.dma_start(scratch[:, sp * SW:(sp + 1) * SW], strip_sb[:])

            if stage < 2:
                continue
            # --- masked softmax finale for batch b ---
            esq = fin_pool.tile([P, SQ], F32, tag="esq", name="esq")
            nc.sync.dma_start(esq[:], scratch.rearrange("o (p f) -> (o p) f", p=P))
            if stage < 3:
                outt = fin_pool.tile([P, SQ], F32, tag="outt", name="outt")
                nc.vector.tensor_copy(outt[:], esq[:])
                nc.sync.dma_start(out[b], outt[:])
                continue
            em = fin_pool.tile([P, SQ], F32, tag="em", name="em")
            part = fin_pool.tile([P, 1], F32, tag="part", name="part")
            nc.vector.tensor_mul(em[:], esq[:], keep_sb[:, b * SQ:(b + 1) * SQ])
            nc.vector.reduce_sum(part[:], em[:], axis=mybir.AxisListType.X)
            if stage < 26:  # stage 25: stop after the fused mul+reduce
                outt = fin_pool.tile([P, SQ], F32, tag="outt", name="outt")
                nc.vector.tensor_copy(outt[:], em[:])
                nc.sync.dma_start(out[b], outt[:])
                continue
            tot_ps = sm_pool.tile([P, 1], F32, tag="small", name="tot_ps")
            nc.tensor.matmul(tot_ps[:], ones_sb[:], part[:], start=True, stop=True)
            recip = fin_pool.tile([P, 1], F32, tag="recip", name="recip")
            nc.vector.reciprocal(recip[:], tot_ps[:])
            if stage < 27:  # stage 26: stop after partition-sum + reciprocal
                outt = fin_pool.tile([P, SQ], F32, tag="outt", name="outt")
                nc.vector.tensor_copy(outt[:], em[:])
                nc.vector.tensor_copy(outt[:, 0:1], recip[:])
                nc.sync.dma_start(out[b], outt[:])
                continue
            outt = fin_pool.tile([P, SQ], F32, tag="outt", name="outt")
            nc.vector.tensor_scalar_mul(outt[:], em[:], recip[:, 0:1])
            nc.sync.dma_start(out[b], outt[:])

    nc.compile()
    return nc


def get_program(stage=None):
    import os

    if stage is None:
        stage = int(os.environ.get("KERNEL_STAGE", "27"))
    if stage not in _CACHE:
        _CACHE[stage] = _build_program(stage)
    return _CACHE[stage]


def prepare_in_maps(memory, target, memory_mask, Wq, Wk, v):
    memory = np.asarray(memory, dtype=np.float32)
    target = np.asarray(target, dtype=np.float32)
    Wq = np.asarray(Wq, dtype=np.float32)
    Wk = np.asarray(Wk, dtype=np.float32)
    v = np.asarray(v, dtype=np.float32)
    mask = np.asarray(memory_mask)

    # host-side sharding / layout prep
    memT = np.ascontiguousarray(memory.transpose(0, 2, 1))          # [B, D, S]
    wkT = np.ascontiguousarray(Wk.T)                                # [D, D]
    wqT = np.ascontiguousarray(Wq.T)                                # [D, D]
    tgtT = np.ascontiguousarray(target.T)                           # [D, B]
    vT = np.ascontiguousarray(v.reshape(ET, P).T)                   # [P, ET]
    keep = np.ascontiguousarray(
        (~mask).astype(np.float32).reshape(B, P, SQ))               # [B, P, SQ]

    return [
        {
            "memT": np.ascontiguousarray(memT[c * NB:(c + 1) * NB]),
            "wkT": wkT,
            "wqT": wqT,
            "tgtT": np.ascontiguousarray(tgtT[:, c * NB:(c + 1) * NB]),
            "vT": vT,
            "keep": np.ascontiguousarray(keep[c * NB:(c + 1) * NB]),
        }
        for c in range(N_CORES)
    ]


def gather_output(results):
    out = np.empty((B, S), dtype=np.float32)
    for c in range(N_CORES):
        out[c * NB:(c + 1) * NB] = results[c]["out"].reshape(NB, S)
    return out


def kernel(memory, target, memory_mask, Wq, Wk, v):
    from concourse.bass_utils import run_bass_kernel_spmd

    in_maps = prepare_in_maps(memory, target, memory_mask, Wq, Wk, v)
    nc = get_program()
    res = run_bass_kernel_spmd(nc, in_maps, list(range(N_CORES)))
    return gather_output(res.results)


# revision 33
# speedup vs baseline: 1.1542x; 1.0867x over previous
"""Additive (Bahdanau) attention scoring kernel for Trainium2, 8-core SPMD.

Reference computation (B=16, S=4096, D=1024, all fp32):
    q      = target @ Wq.T                    # [B, D]
    k      = memory @ Wk.T                    # [B, S, D]
    scores = tanh(q[:, None, :] + k) @ v      # [B, S]
    out    = softmax(scores - 1e9 * mask, axis=-1)

Sharding: batch across the 8 cores (2 batches per core), weights replicated.

Host-side prep (layout only, no math): memory is transposed to [D, S] per
batch so the contraction dim lands on SBUF partitions, and its columns are
compacted to just the unmasked positions (padded with duplicates of the
first kept column to a 128-multiple, tail strip >= 256). Masked positions
contribute exactly 0 to the reference softmax (exp(-1e9) == 0 in fp32), so
skipping their k-matmul columns is algebraically exact.

Per-core device pipeline (python-unrolled, Tile-scheduled):
  - q^T via fp32r matmuls with target as the M=2 stationary and WqT as the
    N=512 moving operand (fp32r hard-faults the device for small moving N),
    transposed into per-partition bias layout through a DRAM bounce.
  - k^T tiles [e=128, s'=w] = WkT chunk.T @ memC chunk, fp32r accumulated
    over d. fp32r operands must be produced by a rounding compute op, so
    every DMA-landed operand gets a DVE cast into a separate f32r tile.
  - One ACT pass fuses the q-add and tanh (q as per-partition bias),
    writing f32r.
  - v-dot on the PE: psum[1, w] += v_chunk.T @ tanh_tile over the 8
    e-chunks; exp() applied in the ACT copy out of PSUM.
  - The exp strip is scattered back to full-S positions on device
    (DRAM bounce to [128, w/128], then indirect DMAs; duplicate pad
    indices are idempotent). scratch_full is zero-filled per batch, so
    masked positions are exactly 0.
  - Softmax finale per batch (no max-shift needed: |scores| <= sum|v| ~ 8,
    exp cannot overflow): [128, 32] esq load, mask multiply, free-dim
    reduce, ones-matmul partition reduce, reciprocal, per-partition scale.
"""

import os
from contextlib import ExitStack

import numpy as np

import concourse.tile as tile
from concourse import bacc, mybir
import concourse.bass as bass

B, S, D = 16, 4096, 1024
N_CORES = 8
NB = B // N_CORES  # batches per core
P = 128
DC = D // P        # contraction chunks
ET = D // P        # e tiles
SW = 512           # full strip width along compacted s
SQ = S // P        # 32: free dim of the [128, 32] softmax layout

F32 = mybir.dt.float32
F32R = mybir.dt.float32r
U32 = mybir.dt.uint32
AF = mybir.ActivationFunctionType

_CACHE = {}


def strip_widths(max_kept):
    """Strip widths covering max_kept compacted columns: full 512-wide strips
    plus a 128-granular tail of at least 256 (small moving-N fp32r matmuls
    hard-fault the device)."""
    total = max(512, ((max_kept + 127) // 128) * 128)
    widths = [SW] * (total // SW)
    rem = total % SW
    if rem:
        widths.append(max(256, rem))
    return tuple(widths)


def _build_program(stage, widths):
    """stage: 1 = dma+matmul+tanh only, 2 = +vdot/exp/scatter, 27 = full."""
    s_pad = sum(widths)
    nslot = s_pad // P  # indirect-scatter slots per batch

    nc = bacc.Bacc("TRN2", target_bir_lowering=False, debug=False)

    memC = nc.dram_tensor("memC", [NB, D, s_pad], F32, kind="ExternalInput").ap()
    wkT = nc.dram_tensor("wkT", [D, D], F32, kind="ExternalInput").ap()
    wqT = nc.dram_tensor("wqT", [D, D], F32, kind="ExternalInput").ap()
    tgtT = nc.dram_tensor("tgtT", [D, NB], F32, kind="ExternalInput").ap()
    vT = nc.dram_tensor("vT", [P, ET], F32, kind="ExternalInput").ap()
    keep = nc.dram_tensor("keep", [NB, P, SQ], F32, kind="ExternalInput").ap()
    idxs = nc.dram_tensor("idxs", [NB, nslot, P], U32, kind="ExternalInput").ap()
    out = nc.dram_tensor("out", [NB, P, SQ], F32, kind="ExternalOutput").ap()

    with tile.TileContext(nc) as tc, ExitStack() as ctx:
        consts = ctx.enter_context(tc.tile_pool(name="consts", bufs=1))
        mem_pool = ctx.enter_context(tc.tile_pool(name="mem", bufs=2))
        tt_pool = ctx.enter_context(tc.tile_pool(name="tt", bufs=4))
        strip_pool = ctx.enter_context(tc.tile_pool(name="strip", bufs=2))
        fin_pool = ctx.enter_context(tc.tile_pool(name="fin", bufs=2))
        kps_pool = ctx.enter_context(tc.tile_pool(name="kps", bufs=4, space="PSUM"))
        vd_pool = ctx.enter_context(tc.tile_pool(name="vd", bufs=2, space="PSUM"))
        sm_pool = ctx.enter_context(tc.tile_pool(name="smps", bufs=2, space="PSUM"))
        dram_pool = ctx.enter_context(tc.tile_pool(name="scratch", bufs=2, space="DRAM"))

        # --- small constants (cheap DMAs first) ---
        tgt_sb = consts.tile([P, DC * NB], F32)
        for dc in range(DC):
            nc.sync.dma_start(tgt_sb[:, dc * NB:(dc + 1) * NB], tgtT[dc * P:(dc + 1) * P, :])
        tgt_r = consts.tile([P, DC * NB], F32R)
        nc.vector.tensor_copy(tgt_r[:], tgt_sb[:])
        v_sb = consts.tile([P, ET], F32)
        nc.sync.dma_start(v_sb[:], vT[:, :])
        v_r = consts.tile([P, ET], F32R)
        nc.vector.tensor_copy(v_r[:], v_sb[:])
        keep_sb = consts.tile([P, NB * SQ], F32)
        for b in range(NB):
            nc.sync.dma_start(keep_sb[:, b * SQ:(b + 1) * SQ], keep[b])
        idx_sb = consts.tile([P, NB * nslot], U32)
        for b in range(NB):
            nc.sync.dma_start(
                idx_sb[:, b * nslot:(b + 1) * nslot],
                idxs[b].rearrange("slot p -> p slot"),
            )
        ones_sb = consts.tile([P, P], F32)
        nc.vector.memset(ones_sb[:], 1.0)
        zero_sb = consts.tile([P, SQ], F32)
        nc.vector.memset(zero_sb[:], 0.0)

        # --- weights: Wq first (the q matmuls below are first in PE order),
        # then Wk. The two f32 landing buffers share one pool slot (their
        # lifetimes are sequential) to stay inside SBUF.
        wq_r = consts.tile([P, DC * D], F32R)
        wq_sb = consts.tile([P, DC * D], F32, tag="wstage", name="wq_sb")
        for dc in range(DC):
            nc.sync.dma_start(wq_sb[:, dc * D:(dc + 1) * D], wqT[dc * P:(dc + 1) * P, :])
            nc.vector.tensor_copy(wq_r[:, dc * D:(dc + 1) * D], wq_sb[:, dc * D:(dc + 1) * D])
        wk_r = consts.tile([P, DC * D], F32R)
        wk_sb = consts.tile([P, DC * D], F32, tag="wstage", name="wk_sb")
        for dc in range(DC):
            nc.sync.dma_start(wk_sb[:, dc * D:(dc + 1) * D], wkT[dc * P:(dc + 1) * P, :])
            nc.vector.tensor_copy(wk_r[:, dc * D:(dc + 1) * D], wk_sb[:, dc * D:(dc + 1) * D])

        q_sb = consts.tile([P, NB * ET], F32)

        # q[b, e] = sum_d target[b, d] * Wq[e, d]: fp32r with target as the
        # M=2 stationary and WqT as the N=512 moving operand. The [2, 1024]
        # result is transposed into per-partition bias layout [128, 16]
        # (b-major columns) through a DRAM bounce.
        q_row = consts.tile([NB, D], F32)
        for j in range(D // SW):
            q_ps2 = sm_pool.tile([NB, SW], F32, tag="small", name="q_ps2")
            for dc in range(DC):
                nc.tensor.matmul(
                    q_ps2[:],
                    tgt_r[:, dc * NB:(dc + 1) * NB],
                    wq_r[:, dc * D + j * SW: dc * D + (j + 1) * SW],
                    start=(dc == 0),
                    stop=(dc == DC - 1),
                )
            nc.vector.tensor_copy(q_row[:, j * SW:(j + 1) * SW], q_ps2[:])
        qscr = dram_pool.tile([NB, D], F32, tag="qscr", name="qscr")
        nc.sync.dma_start(qscr[:], q_row[:])
        for b in range(NB):
            nc.sync.dma_start(
                q_sb[:, b * ET:(b + 1) * ET],
                qscr[b].rearrange("(et p) -> p et", p=P),
            )

        def emit_vd(vd_ps, tts, c, w):
            nc.tensor.matmul(
                vd_ps[:, :w],
                v_r[:, c:c + 1],
                tts[c][:, :w],
                start=(c == 0),
                stop=(c == ET - 1),
            )

        for b in range(NB):
            scratch_full = dram_pool.tile([1, S], F32, tag="scrf", name="scrf")
            # zero-fill: masked positions are never scattered to
            nc.sync.dma_start(
                scratch_full.rearrange("o (p f) -> (o p) f", p=P), zero_sb[:]
            )
            off = 0
            slot0 = 0
            for sp, w in enumerate(widths):
                mem_sb = mem_pool.tile([P, DC * SW], F32)
                mem_r = mem_pool.tile([P, DC * SW], F32R, tag="mem_r", name="mem_r")
                for dc in range(DC):
                    nc.sync.dma_start(
                        mem_sb[:, dc * SW:dc * SW + w],
                        memC[b, dc * P:(dc + 1) * P, off:off + w],
                    )
                    nc.vector.tensor_copy(
                        mem_r[:, dc * SW:dc * SW + w], mem_sb[:, dc * SW:dc * SW + w]
                    )
                vd_ps = vd_pool.tile([1, SW], F32, tag="vd", name="vd_ps")
                tts = []
                for et in range(ET):
                    k_ps = kps_pool.tile([P, SW], F32, tag="k", name="k_ps")
                    for dc in range(DC):
                        nc.tensor.matmul(
                            k_ps[:, :w],
                            wk_r[:, dc * D + et * P: dc * D + (et + 1) * P],
                            mem_r[:, dc * SW:dc * SW + w],
                            start=(dc == 0),
                            stop=(dc == DC - 1),
                        )
                    tt = tt_pool.tile([P, SW], F32R, tag="tt", name="tt")
                    nc.scalar.activation(
                        tt[:, :w], k_ps[:, :w], AF.Tanh,
                        bias=q_sb[:, b * ET + et: b * ET + et + 1],
                    )
                    tts.append(tt)
                    # keep the PE stream 2 e-tiles ahead of the v-dot so it
                    # never stalls waiting on the ACT tanh
                    if stage >= 2 and et >= 2:
                        emit_vd(vd_ps, tts, et - 2, w)
                if stage < 2:
                    if sp == len(widths) - 1:
                        dbg = fin_pool.tile([P, SQ], F32, tag="outt", name="dbg")
                        nc.vector.tensor_copy(dbg[:], tts[7][:, :SQ])
                        nc.sync.dma_start(out[b], dbg[:])
                    off += w
                    slot0 += w // P
                    continue
                emit_vd(vd_ps, tts, ET - 2, w)
                emit_vd(vd_ps, tts, ET - 1, w)

                strip_sb = strip_pool.tile([1, SW], F32, tag="strip", name="strip_sb")
                nc.scalar.activation(strip_sb[:, :w], vd_ps[:, :w], AF.Exp)
                scratch_c = dram_pool.tile([1, SW], F32, tag="scrc", name="scrc")
                nc.sync.dma_start(scratch_c[:, :w], strip_sb[:, :w])
                nslot_w = w // P
                sc_sb = strip_pool.tile([P, SW // P], F32, tag="scsb", name="sc_sb")
                nc.sync.dma_start(
                    sc_sb[:, :nslot_w],
                    scratch_c[:, :w].rearrange("o (p f) -> (o p) f", f=nslot_w),
                )
                for jj in range(nslot_w):
                    col = b * nslot + slot0 + jj
                    nc.gpsimd.indirect_dma_start(
                        out=scratch_full.rearrange("o (s w2) -> (o s) w2", w2=1),
                        out_offset=bass.IndirectOffsetOnAxis(
                            ap=idx_sb[:, col:col + 1], axis=0
                        ),
                        in_=sc_sb[:, jj:jj + 1],
                        in_offset=None,
                    )
                off += w
                slot0 += nslot_w

            if stage < 2:
                continue
            # --- masked softmax finale for batch b ---
            esq = fin_pool.tile([P, SQ], F32, tag="esq", name="esq")
            nc.sync.dma_start(esq[:], scratch_full.rearrange("o (p f) -> (o p) f", p=P))
            if stage < 25:
                outt = fin_pool.tile([P, SQ], F32, tag="outt", name="outt")
                nc.vector.tensor_copy(outt[:], esq[:])
                nc.sync.dma_start(out[b], outt[:])
                continue
            em = fin_pool.tile([P, SQ], F32, tag="em", name="em")
            part = fin_pool.tile([P, 1], F32, tag="part", name="part")
            nc.vector.tensor_mul(em[:], esq[:], keep_sb[:, b * SQ:(b + 1) * SQ])
            nc.vector.reduce_sum(part[:], em[:], axis=mybir.AxisListType.X)
            if stage < 26:
                outt = fin_pool.tile([P, SQ], F32, tag="outt", name="outt")
                nc.vector.tensor_copy(outt[:], em[:])
                nc.sync.dma_start(out[b], outt[:])
                continue
            tot_ps = sm_pool.tile([P, 1], F32, tag="small", name="tot_ps")
            nc.tensor.matmul(tot_ps[:], ones_sb[:], part[:], start=True, stop=True)
            recip = fin_pool.tile([P, 1], F32, tag="recip", name="recip")
            nc.vector.reciprocal(recip[:], tot_ps[:])
            outt = fin_pool.tile([P, SQ], F32, tag="outt", name="outt")
            nc.vector.tensor_scalar_mul(outt[:], em[:], recip[:, 0:1])
            nc.sync.dma_start(out[b], outt[:])

    nc.compile()
    return nc


def get_program(stage=None, widths=None):
    if stage is None:
        stage = int(os.environ.get("KERNEL_STAGE", "27"))
    assert widths is not None
    key = (stage, widths)
    if key not in _CACHE:
        _CACHE[key] = _build_program(stage, widths)
    return _CACHE[key]


def prepare_in_maps(memory, target, memory_mask, Wq, Wk, v):
    memory = np.asarray(memory, dtype=np.float32)
    target = np.asarray(target, dtype=np.float32)
    Wq = np.asarray(Wq, dtype=np.float32)
    Wk = np.asarray(Wk, dtype=np.float32)
    v = np.asarray(v, dtype=np.float32)
    mask = np.asarray(memory_mask)

    # host-side sharding / layout prep (no arithmetic)
    keep_bool = ~mask                                                # [B, S]
    widths = strip_widths(int(keep_bool.sum(1).max()))
    s_pad = sum(widths)

    memT = memory.transpose(0, 2, 1)                                 # [B, D, S] view
    kept_pad = np.empty((B, s_pad), dtype=np.int64)
    for b in range(B):
        k = np.flatnonzero(keep_bool[b])
        kept_pad[b, :len(k)] = k
        kept_pad[b, len(k):] = k[0]  # duplicate pad: idempotent scatter
    memC = np.empty((B, D, s_pad), dtype=np.float32)
    for b in range(B):
        memC[b] = memT[b][:, kept_pad[b]]

    # scatter index slots: strip of width w at compact offset `off` is
    # bounced to [128, w/128] with local position p*(w/128)+jj, so slot
    # (strip, jj) holds kept_pad[off + p*(w/128) + jj] at partition p
    slot_list = []
    off = 0
    for w in widths:
        f = w // P
        block = kept_pad[:, off:off + w].reshape(B, P, f)            # [B, p, jj]
        for jj in range(f):
            slot_list.append(block[:, :, jj])
        off += w
    idxs = np.stack(slot_list, axis=1).astype(np.uint32)             # [B, nslot, P]

    wkT = np.ascontiguousarray(Wk.T)                                 # [D, D]
    wqT = np.ascontiguousarray(Wq.T)                                 # [D, D]
    tgtT = np.ascontiguousarray(target.T)                            # [D, B]
    vT = np.ascontiguousarray(v.reshape(ET, P).T)                    # [P, ET]
    keep = np.ascontiguousarray(
        keep_bool.astype(np.float32).reshape(B, P, SQ))              # [B, P, SQ]

    in_maps = [
        {
            "memC": np.ascontiguousarray(memC[c * NB:(c + 1) * NB]),
            "wkT": wkT,
            "wqT": wqT,
            "tgtT": np.ascontiguousarray(tgtT[:, c * NB:(c + 1) * NB]),
            "vT": vT,
            "keep": np.ascontiguousarray(keep[c * NB:(c + 1) * NB]),
            "idxs": np.ascontiguousarray(idxs[c * NB:(c + 1) * NB]),
        }
        for c in range(N_CORES)
    ]
    return in_maps, widths


def gather_output(results):
    out = np.empty((B, S), dtype=np.float32)
    for c in range(N_CORES):
        out[c * NB:(c + 1) * NB] = results[c]["out"].reshape(NB, S)
    return out


def kernel(memory, target, memory_mask, Wq, Wk, v):
    from concourse.bass_utils import run_bass_kernel_spmd

    in_maps, widths = prepare_in_maps(memory, target, memory_mask, Wq, Wk, v)
    nc = get_program(widths=widths)
    res = run_bass_kernel_spmd(nc, in_maps, list(range(N_CORES)))
    return gather_output(res.results)
